# revision 18
# baseline (speedup 1.0000x reference)
"""BagNet (3x3-patch CNN + FC) Trainium2 Bass kernel, 8-core data parallel.

Network (per patch, 28x28):
  conv1 1->64 3x3 VALID + BN + ReLU   -> (64, 26, 26)
  conv2 64->64 3x3 VALID + BN + ReLU  -> (64, 24, 24)
  conv3 64->128 3x3 VALID + BN + ReLU -> (128, 22, 22)
  FC 61952 -> 100
Input x (128, 1, 84, 84) -> 3x3 grid of 28x28 patches; batch*9 = 1152 patches
sharded 144/core over 8 cores. BN folded into conv weights + bias on host.

Layouts (per core):
  xt  [128, 144*28]   x-pixel on partitions, 3 replicas at partition 0/32/64
                      with y-shift r baked in (replica r holds x[.., y+r]),
                      free = (patch, y). One K=92 matmul per output-x-chunk h
                      contracts all 3 y-taps at once (banded weights).
  A1  [128=(dx,co), (p, y 26, xg 13)]  conv1 out, x-parity split: x = 2*xg+dx.
  A2  [128=(q,co),  (p, y 24, xg 12)]  conv2 out, x-parity split: x = 2*xg+q.
  h3  [128=co3, (p, yo 22, xo 22)]     conv3 out (FC-ready).
conv2 runs 6 K=128/M=128 matmuls per patch: even and odd output parities are
packed into the two M-halves with block-structured weights (2 rhs x-slices x
3 y-taps), accumulating in one PSUM tile. conv3 keeps the pair/single tap
split but batches 2 patches per matmul (N=484, one PSUM bank).
FC streams wf (bf16, host-transposed to [co3, s, co_out]) and accumulates 484
K=128 matmuls into one PSUM bank.
"""
import os
import numpy as np
import ml_dtypes
from contextlib import ExitStack

import concourse.bass as bass
import concourse.tile as tile
from concourse import bacc, mybir
from concourse.bass_utils import run_bass_kernel_spmd

LAST_EXEC_TIME_NS = None
LAST_PROFILE = None

EPS = 1e-5
NCORES = 8
BC = 16          # images per core
P = BC * 9       # patches per core
G = 16           # patches per pipeline group
NG = P // G
CH_S = 11        # wf s-slices per DMA chunk
NCH = 484 // CH_S
DT = mybir.dt.bfloat16
NPDT = ml_dtypes.bfloat16
F32 = mybir.dt.float32
RELU = mybir.ActivationFunctionType.Relu
IDENT = mybir.ActivationFunctionType.Identity


def dedup_ldweights(nc, const_weights=()):
    """Remove InstLdweights whose weights AP equals the previous PE weight
    load (no intervening loads) — the PE array keeps its stationary operand
    across matmuls, so consecutive same-lhsT matmuls only need one load.
    A redundant load carrying sync waits is removed only when its weights
    live in a never-rewritten const tile: the waits then cannot be guarding
    the weight data, so they are migrated onto the next matmul. Resets at
    any non-PE instruction and at block boundaries to stay conservative."""
    from concourse import mybir as cmb
    removed = 0
    for bb in nc.m.functions[0].blocks:
        keep = []
        last_key = None
        pending_waits = []
        changed = False
        for inst in bb.instructions:
            nm = type(inst).__name__
            if nm == "InstLdweights":
                ap = inst.ins[0]
                key = str(ap)
                si = inst.sync_info
                waits = list(si.on_wait) if si else []
                upds = list(si.on_update) if si else []
                tname = str(getattr(ap, "memref", "") or "")
                is_const = any(tname.startswith(c) for c in const_weights)
                if key == last_key and not upds and (not waits or is_const):
                    removed += 1
                    changed = True
                    pending_waits.extend(waits)
                    continue
                last_key = key
            elif nm == "InstMatmult":
                if pending_waits:
                    si = inst.sync_info
                    if si is None:
                        inst.sync_info = cmb.SyncInfo(
                            on_wait=pending_waits, on_update=[])
                    else:
                        si.on_wait = list(si.on_wait) + pending_waits
                    pending_waits = []
            elif getattr(inst, "engine", None) == cmb.EngineType.PE:
                # unknown PE instruction (Drain etc.) — reset conservatively
                last_key = None
                assert not pending_waits, "pending waits hit unknown PE inst"
            # non-PE instructions do not disturb the PE array state
            keep.append(inst)
        assert not pending_waits
        if changed:
            bb.instructions[:] = keep
    return removed


def build_program(repeat=1):
    nc = bacc.Bacc("TRN2", target_bir_lowering=False, debug=False)

    xt_d = nc.dram_tensor("xt", [128, P * 28], DT, kind="ExternalInput")
    w1_d = nc.dram_tensor("w1sb", [128, 13 * 128], DT, kind="ExternalInput")
    w2_d = nc.dram_tensor("w2ab", [128, 6 * 128], DT, kind="ExternalInput")
    w3p_d = nc.dram_tensor("w3p", [128, 6 * 128], DT, kind="ExternalInput")
    w3s_d = nc.dram_tensor("w3s", [128, 3 * 128], DT, kind="ExternalInput")
    b1_d = nc.dram_tensor("b1v", [128, 1], F32, kind="ExternalInput")
    b2_d = nc.dram_tensor("b2v", [128, 1], F32, kind="ExternalInput")
    b3_d = nc.dram_tensor("b3v", [128, 1], F32, kind="ExternalInput")
    bf_d = nc.dram_tensor("bfv", [100, 1], F32, kind="ExternalInput")
    wf_d = nc.dram_tensor("wfq", [128, 484 * 100], DT, kind="ExternalInput")
    out_d = nc.dram_tensor("out", [100, P], F32, kind="ExternalOutput")

    with tile.TileContext(nc) as tc, ExitStack() as ctx:
        const = ctx.enter_context(tc.tile_pool(name="const", bufs=1))
        a1p = ctx.enter_context(tc.tile_pool(name="a1", bufs=2))
        a2p = ctx.enter_context(tc.tile_pool(name="a2", bufs=2))
        h3p = ctx.enter_context(tc.tile_pool(name="h3", bufs=1))
        wfp = ctx.enter_context(tc.tile_pool(name="wf", bufs=6))
        ps = ctx.enter_context(tc.tile_pool(name="ps", bufs=7, space="PSUM"))
        fcp = ctx.enter_context(tc.tile_pool(name="fcps", bufs=1, space="PSUM"))

        xt = const.tile([128, P, 28], DT)
        nc.sync.dma_start(xt[:], xt_d[:].rearrange("a (p y) -> a p y", y=28))
        w1sb = const.tile([128, 13 * 128], DT)
        nc.sync.dma_start(w1sb[:], w1_d[:])
        w2ab = const.tile([128, 6 * 128], DT)
        nc.sync.dma_start(w2ab[:], w2_d[:])
        w3pr = const.tile([128, 6 * 128], DT)
        nc.sync.dma_start(w3pr[:], w3p_d[:])
        w3sg = const.tile([128, 3 * 128], DT)
        nc.sync.dma_start(w3sg[:], w3s_d[:])
        b1t = const.tile([128, 1], F32)
        nc.sync.dma_start(b1t[:], b1_d[:])
        b2t = const.tile([128, 1], F32)
        nc.sync.dma_start(b2t[:], b2_d[:])
        b3t = const.tile([128, 1], F32)
        nc.sync.dma_start(b3t[:], b3_d[:])
        bft = const.tile([100, 1], F32)
        nc.sync.dma_start(bft[:], bf_d[:])

        h3 = h3p.tile([128, P, 22, 22], DT)
        fc_psum = fcp.tile([128, P], F32)

        def conv1(gs):
            """conv1 for a list of groups, h-outer so same-h matmuls share
            one weight load. Generator: allocates A1 tiles eagerly (first
            next()), then emits one h-step per next() so the caller can
            interleave conv1 work into other matmul streams."""
            a1s = {}
            for g in gs:
                a1s[g] = a1p.tile([128, G, 26, 13], DT, tag="A1", name=f"A1_{g}")
            yield a1s
            for h in range(13):
                c1s = {}
                for g in gs:
                    c1 = ps.tile([128, G, 26], F32, tag="cps", name=f"c1_{g}_{h}")
                    nc.tensor.matmul(
                        c1[:, :, :],
                        w1sb[0:92, h * 128:(h + 1) * 128],
                        xt[0:92, g * G:(g + 1) * G, 0:26],
                        start=True, stop=True,
                    )
                    c1s[g] = c1
                for j, g in enumerate(gs):
                    if (h + j) % 2 == 0:
                        nc.scalar.activation(a1s[g][:, :, :, h], c1s[g][:, :, :],
                                             RELU, bias=b1t[:, :], scale=1.0)
                    else:
                        nc.vector.tensor_scalar(a1s[g][:, :, :, h],
                                                c1s[g][:, :, :], b1t[:, :], 0.0,
                                                mybir.AluOpType.add,
                                                mybir.AluOpType.max)
                yield None
            return a1s

        def conv2(A1, A2):
            """conv2 tap-outer over 4-patch blocks: 6 weight loads/block."""
            for pb in range(0, G, 4):
                c2s = [ps.tile([128, 24, 12], F32, tag="cps", name=f"c2_{pb}_{i}")
                       for i in range(4)]
                for t in range(6):
                    ki, b = t // 2, t % 2
                    yv = slice(ki, ki + 24)
                    wv = w2ab[:, (3 * b + ki) * 128:(3 * b + ki + 1) * 128]
                    for i in range(4):
                        nc.tensor.matmul(
                            c2s[i][:, :, :], wv,
                            A1[:, pb + i, yv, b:b + 12],
                            start=(t == 0), stop=(t == 5))
                for i in range(4):
                    if i % 2 == 0:
                        nc.scalar.activation(A2[:, pb + i], c2s[i][:, :, :],
                                             RELU, bias=b2t[:, :], scale=1.0)
                    else:
                        nc.vector.tensor_scalar(A2[:, pb + i], c2s[i][:, :, :],
                                                b2t[:, :], 0.0,
                                                mybir.AluOpType.add,
                                                mybir.AluOpType.max)

        def conv3(A2, g, filler=None):
            """conv3 tap-outer over 2 patch-pair blocks (4 patches):
            12 weight loads/block; singles alternate row halves. After each
            block, drains a few steps from `filler` (conv1 of the next
            pair) to keep the PE fed while this block's psums drain."""
            def drain(n):
                if filler is None:
                    return
                for _ in range(n):
                    if next(filler, StopIteration) is StopIteration:
                        break
            for pb in range(0, G, 4):
                ce = [ps.tile([128, 2, 22, 11], F32, tag="cps", name=f"c3e_{pb}_{i}")
                      for i in range(2)]
                co = [ps.tile([128, 2, 22, 11], F32, tag="cps", name=f"c3o_{pb}_{i}")
                      for i in range(2)]
                pvs = [slice(pb + 2 * i, pb + 2 * i + 2) for i in range(2)]
                for ki in range(3):
                    yv = slice(ki, ki + 22)
                    we = w3pr[:, ki * 128:(ki + 1) * 128]
                    wo = w3pr[:, (3 + ki) * 128:(4 + ki) * 128]
                    for i in range(2):
                        nc.tensor.matmul(ce[i][:], we, A2[:, pvs[i], yv, 0:11],
                                         start=(ki == 0), stop=False)
                    for i in range(2):
                        nc.tensor.matmul(co[i][:], wo, A2[:, pvs[i], yv, 1:12],
                                         start=(ki == 0), stop=False)
                for ki in range(3):
                    yv = slice(ki, ki + 22)
                    we = w3sg[0:64, ki * 128:(ki + 1) * 128]
                    wo = w3sg[64:128, ki * 128:(ki + 1) * 128]
                    for i in range(2):
                        nc.tensor.matmul(ce[i][:], we, A2[0:64, pvs[i], yv, 1:12],
                                         start=False, stop=(ki == 2))
                        nc.tensor.matmul(co[i][:], wo, A2[64:128, pvs[i], yv, 0:11],
                                         start=False, stop=(ki == 2))
                for i in range(2):
                    pg = g * G + pb + 2 * i
                    pgv = slice(pg, pg + 2)
                    nc.scalar.activation(h3[:, pgv, :, 0:22:2], ce[i][:], RELU,
                                         bias=b3t[:, :], scale=1.0)
                    nc.vector.tensor_scalar(h3[:, pgv, :, 1:22:2], co[i][:],
                                            b3t[:, :], 0.0,
                                            mybir.AluOpType.add,
                                            mybir.AluOpType.max)
                drain(4)

        rep_ctx = tc.For_i(0, repeat, 1) if repeat > 1 else None
        if rep_ctx is not None:
            rep_ctx.__enter__()
        pairs = [[g for g in (g0, g0 + 1) if g < NG] for g0 in range(0, NG, 2)]
        gen = conv1(pairs[0])
        a1s = dict(next(gen))
        for _ in gen:
            pass
        for idx, gs in enumerate(pairs):
            for j, g in enumerate(gs):
                A2 = a2p.tile([128, G, 24, 12], DT, tag="A2", name=f"A2_{g}")
                conv2(a1s[g], A2)
                filler = None
                if j == len(gs) - 1 and idx + 1 < len(pairs):
                    filler = conv1(pairs[idx + 1])
                    a1s.update(next(filler))
                conv3(A2, g, filler=filler)
                if filler is not None:
                    for _ in filler:
                        pass

        # ---- FC: 484 accumulating K=128 matmuls, wf streamed ----
        for c in range(NCH):
            wfb = wfp.tile([128, CH_S * 100], DT)
            eng = nc.sync if c % 2 == 0 else nc.gpsimd
            eng.dma_start(wfb[:], wf_d[:, c * CH_S * 100:(c + 1) * CH_S * 100])
            for sl in range(CH_S):
                s = c * CH_S + sl
                nc.tensor.matmul(
                    fc_psum[0:100, :], wfb[:, sl * 100:(sl + 1) * 100],
                    h3[:, :, s // 22, s % 22],
                    start=(s == 0), stop=(s == 483))
        outb = const.tile([100, P], F32)
        nc.scalar.activation(outb[:], fc_psum[0:100, :], IDENT,
                             bias=bft[:, :], scale=1.0)
        nc.sync.dma_start(out_d[:], outb[:])
        if rep_ctx is not None:
            rep_ctx.__exit__(None, None, None)

    dedup_ldweights(nc, const_weights=("w1sb", "w2ab", "w3pr", "w3sg"))
    nc.compile()
    return nc


def _fold_bn(w, b, g_, be, m, v):
    s = (g_ / np.sqrt(v + EPS)).astype(np.float32)
    return (w * s[:, None, None, None]).astype(np.float32), \
           (be - (m - b) * s).astype(np.float32)


def prep_shared(inputs):
    """Host-side: fold BN, build weight layouts shared by all cores."""
    f = np.float32
    w1f, b1f = _fold_bn(inputs["w1"], inputs["b1"], inputs["g1"], inputs["be1"],
                        inputs["m1"], inputs["v1"])
    w2f, b2f = _fold_bn(inputs["w2"], inputs["b2"], inputs["g2"], inputs["be2"],
                        inputs["m2"], inputs["v2"])
    w3f, b3f = _fold_bn(inputs["w3"], inputs["b3"], inputs["g3"], inputs["be3"],
                        inputs["m3"], inputs["v3"])

    # conv1 banded: lhsT[32r+x, h*128 + dx*64+co] = w1f[co, r, x-(2h+dx)]
    w1r = w1f[:, 0]                      # (64, ki, kj)
    w1sb = np.zeros((128, 13 * 128), f)
    for h in range(13):
        for r in range(3):
            for dx in range(2):
                for kj in range(3):
                    x = 2 * h + dx + kj
                    w1sb[32 * r + x, h * 128 + dx * 64:h * 128 + (dx + 1) * 64] \
                        = w1r[:, r, kj]

    # conv2 merged-parity blocks: A (x-slice 0) and B (x-slice +1) per ki
    w2ab = np.zeros((128, 6 * 128), f)
    for ki in range(3):
        A = ki * 128
        B = (3 + ki) * 128
        w2ab[0:64, A:A + 64] = w2f[:, :, ki, 0].T       # d0 -> even
        w2ab[64:128, A:A + 64] = w2f[:, :, ki, 1].T     # d1 -> even
        w2ab[64:128, A + 64:A + 128] = w2f[:, :, ki, 0].T   # d1 -> odd
        w2ab[0:64, B:B + 64] = w2f[:, :, ki, 2].T       # d0 -> even
        w2ab[0:64, B + 64:B + 128] = w2f[:, :, ki, 1].T     # d0 -> odd
        w2ab[64:128, B + 64:B + 128] = w2f[:, :, ki, 2].T   # d1 -> odd

    # conv3 pair/single tap tiles (unchanged structure)
    pr = np.zeros((128, 6 * 128), f)
    sg = np.zeros((128, 3 * 128), f)
    for ki in range(3):
        pr[0:64, ki * 128:(ki + 1) * 128] = w3f[:, :, ki, 0].T
        pr[64:128, ki * 128:(ki + 1) * 128] = w3f[:, :, ki, 1].T
        pr[0:64, (3 + ki) * 128:(4 + ki) * 128] = w3f[:, :, ki, 1].T
        pr[64:128, (3 + ki) * 128:(4 + ki) * 128] = w3f[:, :, ki, 2].T
        sg[0:64, ki * 128:(ki + 1) * 128] = w3f[:, :, ki, 2].T
        sg[64:128, ki * 128:(ki + 1) * 128] = w3f[:, :, ki, 0].T

    wfq = np.ascontiguousarray(
        inputs["wf"].astype(f).reshape(100, 128, 484).transpose(1, 2, 0)
    ).reshape(128, 484 * 100)

    return {
        "w1sb": w1sb.astype(NPDT),
        "w2ab": w2ab.astype(NPDT),
        "w3p": pr.astype(NPDT), "w3s": sg.astype(NPDT),
        "b1v": np.tile(b1f, 2)[:, None].astype(f),
        "b2v": np.tile(b2f, 2)[:, None].astype(f),
        "b3v": b3f[:, None].astype(f),
        "bfv": inputs["bf"].astype(f)[:, None],
        "wfq": wfq.astype(NPDT),
    }


def prep_core(x, c):
    """Per-core input: x-pixel-on-partition patches, 3 y-shifted replicas."""
    xs = np.asarray(x)[c * BC:(c + 1) * BC, 0].astype(np.float32)  # (16,84,84)
    xr = xs.reshape(BC, 3, 28, 3, 28).transpose(4, 0, 1, 3, 2)     # (x,b,hb,wb,y)
    xrf = xr.reshape(28, P, 28)
    xt = np.zeros((128, P, 28), NPDT)
    for r in range(3):
        xt[32 * r:32 * r + 28, :, 0:28 - r] = xrf[:, :, r:].astype(NPDT)
    return {"xt": xt.reshape(128, P * 28)}


def bench(inputs, iters=8, repeat=192):
    """Measure per-iteration HW time by running the kernel body `repeat`
    times inside one dispatch (tc.For_i) and comparing against repeat=1.
    The ~60-80ms axon dispatch overhead cancels in the difference.
    """
    t1 = _bench_one(inputs, iters, 1)
    tr = _bench_one(inputs, iters, repeat)
    return (tr - t1) / (repeat - 1)


def _bench_one(inputs, iters, repeat):
    import time as _time
    import jax
    from jax.sharding import Mesh, PartitionSpec, NamedSharding
    from jax.experimental.shard_map import shard_map
    from concourse import mybir as _mb
    from concourse import bass2jax

    inputs = {k: np.asarray(v) for k, v in inputs.items()}
    shared = prep_shared(inputs)
    in_maps = [{**shared, **prep_core(inputs["x"], c)} for c in range(NCORES)]
    nc = build_program(repeat=repeat)
    bass2jax.install_neuronx_cc_hook()

    partition_name = nc.partition_id_tensor.name if nc.partition_id_tensor else None
    in_names, out_names, out_avals, zero_outs = [], [], [], []
    for alloc in nc.m.functions[0].allocations:
        if not isinstance(alloc, _mb.MemoryLocationSet):
            continue
        name = alloc.memorylocations[0].name
        if alloc.kind == "ExternalInput":
            if name != partition_name:
                in_names.append(name)
        elif alloc.kind == "ExternalOutput":
            shape = tuple(alloc.tensor_shape)
            dtype = _mb.dt.np(alloc.dtype)
            out_names.append(name)
            out_avals.append(jax.core.ShapedArray(shape, dtype))
            zero_outs.append(np.zeros(shape, dtype))
    n_params = len(in_names)
    all_names = in_names + out_names
    if partition_name is not None:
        all_names = all_names + [partition_name]
    donate = tuple(range(n_params, n_params + len(out_names)))

    def _body(*args):
        operands = list(args)
        if partition_name is not None:
            operands.append(bass2jax.partition_id_tensor())
        outs = bass2jax._bass_exec_p.bind(
            *operands,
            out_avals=tuple(out_avals),
            in_names=tuple(all_names),
            out_names=tuple(out_names),
            lowering_input_output_aliases=(),
            sim_require_finite=True,
            sim_require_nnan=True,
            nc=nc,
        )
        return tuple(outs)

    devices = jax.devices()[:NCORES]
    mesh = Mesh(np.asarray(devices), ("core",))
    spec = NamedSharding(mesh, PartitionSpec("core"))
    sharded = jax.jit(
        shard_map(_body, mesh=mesh,
                  in_specs=(PartitionSpec("core"),) * (n_params + len(out_names)),
                  out_specs=(PartitionSpec("core"),) * len(out_names),
                  check_rep=False),
        donate_argnums=donate, keep_unused=True)

    concat_in = [
        jax.device_put(
            np.concatenate([np.asarray(in_maps[c][n]) for c in range(NCORES)], axis=0),
            spec)
        for n in in_names
    ]

    def _zeros():
        return [jax.device_put(np.zeros((NCORES * z.shape[0], *z.shape[1:]), z.dtype), spec)
                for z in zero_outs]

    r = sharded(*concat_in, *_zeros())   # compile + warm
    jax.block_until_ready(r)
    times = []
    for _ in range(iters):
        zs = _zeros()
        jax.block_until_ready(zs)
        t0 = _time.perf_counter()
        r = sharded(*concat_in, *zs)
        jax.block_until_ready(r)
        times.append(_time.perf_counter() - t0)
    return min(times) * 1e9


def kernel(**inputs):
    global LAST_EXEC_TIME_NS, LAST_PROFILE
    inputs = {k: np.asarray(v) for k, v in inputs.items()}
    shared = prep_shared(inputs)
    in_maps = [{**shared, **prep_core(inputs["x"], c)} for c in range(NCORES)]
    nc = build_program()
    trace = bool(os.environ.get("BASS_KERNEL_TRACE"))
    res = run_bass_kernel_spmd(nc, in_maps, list(range(NCORES)), trace=trace)
    LAST_EXEC_TIME_NS = res.exec_time_ns
    LAST_PROFILE = res.profile_json
    outs = [
        np.asarray(res.results[c]["out"]).T.reshape(BC, 3, 3, 100)
        for c in range(NCORES)
    ]
    return np.concatenate(outs, axis=0)


# revision 19
# speedup vs baseline: 1.1966x; 1.1966x over previous
"""BagNet (3x3-patch CNN + FC) Trainium2 Bass kernel, 8-core data parallel.

Network (per patch, 28x28):
  conv1 1->64 3x3 VALID + BN + ReLU   -> (64, 26, 26)
  conv2 64->64 3x3 VALID + BN + ReLU  -> (64, 24, 24)
  conv3 64->128 3x3 VALID + BN + ReLU -> (128, 22, 22)
  FC 61952 -> 100
Input x (128, 1, 84, 84) -> 3x3 grid of 28x28 patches; batch*9 = 1152 patches
sharded 144/core over 8 cores. BN folded into conv weights + bias on host.

Layouts (per core):
  xt  [128, 144*28]   x-pixel on partitions, 3 replicas at partition 0/32/64
                      with y-shift r baked in (replica r holds x[.., y+r]),
                      free = (patch, y). One K=92 matmul per output-x-chunk h
                      contracts all 3 y-taps at once (banded weights).
  A1  [128=(dx,co), (p, y 26, xg 13)]  conv1 out, x-parity split: x = 2*xg+dx.
  A2  [128=(q,co),  (p, y 24, xg 12)]  conv2 out, x-parity split: x = 2*xg+q.
  h3  [128=co3, (p, yo 22, xo 22)]     conv3 out (FC-ready).
conv2 runs 6 K=128/M=128 matmuls per patch: even and odd output parities are
packed into the two M-halves with block-structured weights (2 rhs x-slices x
3 y-taps), accumulating in one PSUM tile. conv3 keeps the pair/single tap
split but batches 2 patches per matmul (N=484, one PSUM bank).
FC streams wf (bf16, host-transposed to [co3, s, co_out]) and accumulates 484
K=128 matmuls into one PSUM bank.
"""
import os
import numpy as np
import ml_dtypes
from contextlib import ExitStack

import concourse.bass as bass
import concourse.tile as tile
from concourse import bacc, mybir
from concourse.bass_utils import run_bass_kernel_spmd

LAST_EXEC_TIME_NS = None
LAST_PROFILE = None

EPS = 1e-5
NCORES = 8
BC = 16          # images per core
P = BC * 9       # patches per core
G = 16           # patches per pipeline group
NG = P // G
CH_S = 11        # wf s-slices per DMA chunk
NCH = 484 // CH_S
DT = mybir.dt.bfloat16
NPDT = ml_dtypes.bfloat16
F32 = mybir.dt.float32
RELU = mybir.ActivationFunctionType.Relu
IDENT = mybir.ActivationFunctionType.Identity


def dedup_ldweights(nc, const_weights=()):
    """Remove InstLdweights whose weights AP equals the previous PE weight
    load (no intervening loads) — the PE array keeps its stationary operand
    across matmuls, so consecutive same-lhsT matmuls only need one load.
    A redundant load carrying sync waits is removed only when its weights
    live in a never-rewritten const tile: the waits then cannot be guarding
    the weight data, so they are migrated onto the next matmul. Resets at
    any non-PE instruction and at block boundaries to stay conservative."""
    from concourse import mybir as cmb
    removed = 0
    for bb in nc.m.functions[0].blocks:
        keep = []
        last_key = None
        pending_waits = []
        changed = False
        for inst in bb.instructions:
            nm = type(inst).__name__
            if nm == "InstLdweights":
                ap = inst.ins[0]
                key = str(ap)
                si = inst.sync_info
                waits = list(si.on_wait) if si else []
                upds = list(si.on_update) if si else []
                tname = str(getattr(ap, "memref", "") or "")
                is_const = any(tname.startswith(c) for c in const_weights)
                if key == last_key and not upds and (not waits or is_const):
                    removed += 1
                    changed = True
                    pending_waits.extend(waits)
                    continue
                last_key = key
            elif nm == "InstMatmult":
                if pending_waits:
                    si = inst.sync_info
                    if si is None:
                        inst.sync_info = cmb.SyncInfo(
                            on_wait=pending_waits, on_update=[])
                    else:
                        si.on_wait = list(si.on_wait) + pending_waits
                    pending_waits = []
            elif getattr(inst, "engine", None) == cmb.EngineType.PE:
                # unknown PE instruction (Drain etc.) — reset conservatively
                last_key = None
                assert not pending_waits, "pending waits hit unknown PE inst"
            # non-PE instructions do not disturb the PE array state
            keep.append(inst)
        assert not pending_waits
        if changed:
            bb.instructions[:] = keep
    return removed


def build_program(repeat=1):
    nc = bacc.Bacc("TRN2", target_bir_lowering=False, debug=False)

    xt_d = nc.dram_tensor("xt", [128, P * 28], DT, kind="ExternalInput")
    w1_d = nc.dram_tensor("w1sb", [128, 13 * 128], DT, kind="ExternalInput")
    w2_d = nc.dram_tensor("w2ab", [128, 6 * 128], DT, kind="ExternalInput")
    w3p_d = nc.dram_tensor("w3p", [128, 6 * 128], DT, kind="ExternalInput")
    w3s_d = nc.dram_tensor("w3s", [128, 3 * 128], DT, kind="ExternalInput")
    b1_d = nc.dram_tensor("b1v", [128, 1], F32, kind="ExternalInput")
    b2_d = nc.dram_tensor("b2v", [128, 1], F32, kind="ExternalInput")
    b3_d = nc.dram_tensor("b3v", [128, 1], F32, kind="ExternalInput")
    bf_d = nc.dram_tensor("bfv", [100, 1], F32, kind="ExternalInput")
    wf_d = nc.dram_tensor("wfq", [128, 484 * 100], DT, kind="ExternalInput")
    out_d = nc.dram_tensor("out", [100, P], F32, kind="ExternalOutput")

    with tile.TileContext(nc) as tc, ExitStack() as ctx:
        const = ctx.enter_context(tc.tile_pool(name="const", bufs=1))
        a1p = ctx.enter_context(tc.tile_pool(name="a1", bufs=2))
        a2p = ctx.enter_context(tc.tile_pool(name="a2", bufs=2))
        h3p = ctx.enter_context(tc.tile_pool(name="h3", bufs=1))
        wfp = ctx.enter_context(tc.tile_pool(name="wf", bufs=6))
        ps = ctx.enter_context(tc.tile_pool(name="ps", bufs=7, space="PSUM"))
        fcp = ctx.enter_context(tc.tile_pool(name="fcps", bufs=1, space="PSUM"))

        xt = const.tile([128, P, 28], DT)
        nc.sync.dma_start(xt[:], xt_d[:].rearrange("a (p y) -> a p y", y=28))
        w1sb = const.tile([128, 13 * 128], DT)
        nc.sync.dma_start(w1sb[:], w1_d[:])
        w2ab = const.tile([128, 6 * 128], DT)
        nc.sync.dma_start(w2ab[:], w2_d[:])
        w3pr = const.tile([128, 6 * 128], DT)
        nc.sync.dma_start(w3pr[:], w3p_d[:])
        w3sg = const.tile([128, 3 * 128], DT)
        nc.sync.dma_start(w3sg[:], w3s_d[:])
        b1t = const.tile([128, 1], F32)
        nc.sync.dma_start(b1t[:], b1_d[:])
        b2t = const.tile([128, 1], F32)
        nc.sync.dma_start(b2t[:], b2_d[:])
        b3t = const.tile([128, 1], F32)
        nc.sync.dma_start(b3t[:], b3_d[:])
        bft = const.tile([100, 1], F32)
        nc.sync.dma_start(bft[:], bf_d[:])

        h3 = h3p.tile([128, P, 22, 22], DT)
        fc_psum = fcp.tile([128, P], F32)

        def conv1(gs):
            """conv1 for a list of groups, h-outer so same-h matmuls share
            one weight load. Generator: allocates A1 tiles eagerly (first
            next()), then emits one h-step per next() so the caller can
            interleave conv1 work into other matmul streams."""
            a1s = {}
            for g in gs:
                a1s[g] = a1p.tile([128, G, 26, 13], DT, tag="A1", name=f"A1_{g}")
            yield a1s
            for h in range(13):
                c1s = {}
                for g in gs:
                    c1 = ps.tile([128, G, 26], F32, tag="cps", name=f"c1_{g}_{h}")
                    nc.tensor.matmul(
                        c1[:, :, :],
                        w1sb[0:92, h * 128:(h + 1) * 128],
                        xt[0:92, g * G:(g + 1) * G, 0:26],
                        start=True, stop=True,
                    )
                    c1s[g] = c1
                for j, g in enumerate(gs):
                    if (h + j) % 2 == 0:
                        nc.scalar.activation(a1s[g][:, :, :, h], c1s[g][:, :, :],
                                             RELU, bias=b1t[:, :], scale=1.0)
                    else:
                        nc.vector.tensor_scalar(a1s[g][:, :, :, h],
                                                c1s[g][:, :, :], b1t[:, :], 0.0,
                                                mybir.AluOpType.add,
                                                mybir.AluOpType.max)
                yield None
            return a1s

        def conv2(A1, A2):
            """conv2 tap-outer over 4-patch blocks: 6 weight loads/block."""
            for pb in range(0, G, 4):
                c2s = [ps.tile([128, 24, 12], F32, tag="cps", name=f"c2_{pb}_{i}")
                       for i in range(4)]
                order = list(range(6))
                if (pb // 4) % 2 == 1:
                    order.reverse()
                for idx, t in enumerate(order):
                    ki, b = t // 2, t % 2
                    yv = slice(ki, ki + 24)
                    wv = w2ab[:, (3 * b + ki) * 128:(3 * b + ki + 1) * 128]
                    for i in range(4):
                        nc.tensor.matmul(
                            c2s[i][:, :, :], wv,
                            A1[:, pb + i, yv, b:b + 12],
                            start=(idx == 0), stop=(idx == 5))
                for i in range(4):
                    if i % 2 == 0:
                        nc.scalar.activation(A2[:, pb + i], c2s[i][:, :, :],
                                             RELU, bias=b2t[:, :], scale=1.0)
                    else:
                        nc.vector.tensor_scalar(A2[:, pb + i], c2s[i][:, :, :],
                                                b2t[:, :], 0.0,
                                                mybir.AluOpType.add,
                                                mybir.AluOpType.max)

        def conv3(A2, g, filler=None):
            """conv3 tap-outer over 2 patch-pair blocks (4 patches):
            12 weight loads/block; singles alternate row halves. After each
            block, drains a few steps from `filler` (conv1 of the next
            pair) to keep the PE fed while this block's psums drain."""
            def drain(n):
                if filler is None:
                    return
                for _ in range(n):
                    if next(filler, StopIteration) is StopIteration:
                        break
            for pb in range(0, G, 4):
                ce = [ps.tile([128, 2, 22, 11], F32, tag="cps", name=f"c3e_{pb}_{i}")
                      for i in range(2)]
                co = [ps.tile([128, 2, 22, 11], F32, tag="cps", name=f"c3o_{pb}_{i}")
                      for i in range(2)]
                pvs = [slice(pb + 2 * i, pb + 2 * i + 2) for i in range(2)]
                for ki in range(3):
                    yv = slice(ki, ki + 22)
                    we = w3pr[:, ki * 128:(ki + 1) * 128]
                    wo = w3pr[:, (3 + ki) * 128:(4 + ki) * 128]
                    for i in range(2):
                        nc.tensor.matmul(ce[i][:], we, A2[:, pvs[i], yv, 0:11],
                                         start=(ki == 0), stop=False)
                    for i in range(2):
                        nc.tensor.matmul(co[i][:], wo, A2[:, pvs[i], yv, 1:12],
                                         start=(ki == 0), stop=False)
                for ki in range(3):
                    yv = slice(ki, ki + 22)
                    we = w3sg[0:64, ki * 128:(ki + 1) * 128]
                    wo = w3sg[64:128, ki * 128:(ki + 1) * 128]
                    nc.tensor.matmul(ce[0][:], we, A2[0:64, pvs[0], yv, 1:12],
                                     start=False, stop=(ki == 2))
                    nc.tensor.matmul(co[0][:], wo, A2[64:128, pvs[0], yv, 0:11],
                                     start=False, stop=(ki == 2))
                    nc.tensor.matmul(co[1][:], wo, A2[64:128, pvs[1], yv, 0:11],
                                     start=False, stop=(ki == 2))
                    nc.tensor.matmul(ce[1][:], we, A2[0:64, pvs[1], yv, 1:12],
                                     start=False, stop=(ki == 2))
                for i in range(2):
                    pg = g * G + pb + 2 * i
                    pgv = slice(pg, pg + 2)
                    nc.scalar.activation(h3[:, pgv, :, 0:22:2], ce[i][:], RELU,
                                         bias=b3t[:, :], scale=1.0)
                    nc.vector.tensor_scalar(h3[:, pgv, :, 1:22:2], co[i][:],
                                            b3t[:, :], 0.0,
                                            mybir.AluOpType.add,
                                            mybir.AluOpType.max)
                drain(4)

        rep_ctx = tc.For_i(0, repeat, 1) if repeat > 1 else None
        if rep_ctx is not None:
            rep_ctx.__enter__()
        pairs = [[g for g in (g0, g0 + 1) if g < NG] for g0 in range(0, NG, 2)]
        gen = conv1(pairs[0])
        a1s = dict(next(gen))
        for _ in gen:
            pass
        for idx, gs in enumerate(pairs):
            for j, g in enumerate(gs):
                A2 = a2p.tile([128, G, 24, 12], DT, tag="A2", name=f"A2_{g}")
                conv2(a1s[g], A2)
                filler = None
                if j == len(gs) - 1 and idx + 1 < len(pairs):
                    filler = conv1(pairs[idx + 1])
                    a1s.update(next(filler))
                conv3(A2, g, filler=filler)
                if filler is not None:
                    for _ in filler:
                        pass

        # ---- FC: 484 accumulating K=128 matmuls, wf streamed ----
        for c in range(NCH):
            wfb = wfp.tile([128, CH_S * 100], DT)
            eng = nc.sync if c % 2 == 0 else nc.gpsimd
            eng.dma_start(wfb[:], wf_d[:, c * CH_S * 100:(c + 1) * CH_S * 100])
            for sl in range(CH_S):
                s = c * CH_S + sl
                nc.tensor.matmul(
                    fc_psum[0:100, :], wfb[:, sl * 100:(sl + 1) * 100],
                    h3[:, :, s // 22, s % 22],
                    start=(s == 0), stop=(s == 483))
        outb = const.tile([100, P], F32)
        nc.scalar.activation(outb[:], fc_psum[0:100, :], IDENT,
                             bias=bft[:, :], scale=1.0)
        nc.sync.dma_start(out_d[:], outb[:])
        if rep_ctx is not None:
            rep_ctx.__exit__(None, None, None)

    dedup_ldweights(nc, const_weights=("w1sb", "w2ab", "w3pr", "w3sg"))
    nc.compile()
    return nc


def _fold_bn(w, b, g_, be, m, v):
    s = (g_ / np.sqrt(v + EPS)).astype(np.float32)
    return (w * s[:, None, None, None]).astype(np.float32), \
           (be - (m - b) * s).astype(np.float32)


def prep_shared(inputs):
    """Host-side: fold BN, build weight layouts shared by all cores."""
    f = np.float32
    w1f, b1f = _fold_bn(inputs["w1"], inputs["b1"], inputs["g1"], inputs["be1"],
                        inputs["m1"], inputs["v1"])
    w2f, b2f = _fold_bn(inputs["w2"], inputs["b2"], inputs["g2"], inputs["be2"],
                        inputs["m2"], inputs["v2"])
    w3f, b3f = _fold_bn(inputs["w3"], inputs["b3"], inputs["g3"], inputs["be3"],
                        inputs["m3"], inputs["v3"])

    # conv1 banded: lhsT[32r+x, h*128 + dx*64+co] = w1f[co, r, x-(2h+dx)]
    w1r = w1f[:, 0]                      # (64, ki, kj)
    w1sb = np.zeros((128, 13 * 128), f)
    for h in range(13):
        for r in range(3):
            for dx in range(2):
                for kj in range(3):
                    x = 2 * h + dx + kj
                    w1sb[32 * r + x, h * 128 + dx * 64:h * 128 + (dx + 1) * 64] \
                        = w1r[:, r, kj]

    # conv2 merged-parity blocks: A (x-slice 0) and B (x-slice +1) per ki
    w2ab = np.zeros((128, 6 * 128), f)
    for ki in range(3):
        A = ki * 128
        B = (3 + ki) * 128
        w2ab[0:64, A:A + 64] = w2f[:, :, ki, 0].T       # d0 -> even
        w2ab[64:128, A:A + 64] = w2f[:, :, ki, 1].T     # d1 -> even
        w2ab[64:128, A + 64:A + 128] = w2f[:, :, ki, 0].T   # d1 -> odd
        w2ab[0:64, B:B + 64] = w2f[:, :, ki, 2].T       # d0 -> even
        w2ab[0:64, B + 64:B + 128] = w2f[:, :, ki, 1].T     # d0 -> odd
        w2ab[64:128, B + 64:B + 128] = w2f[:, :, ki, 2].T   # d1 -> odd

    # conv3 pair/single tap tiles (unchanged structure)
    pr = np.zeros((128, 6 * 128), f)
    sg = np.zeros((128, 3 * 128), f)
    for ki in range(3):
        pr[0:64, ki * 128:(ki + 1) * 128] = w3f[:, :, ki, 0].T
        pr[64:128, ki * 128:(ki + 1) * 128] = w3f[:, :, ki, 1].T
        pr[0:64, (3 + ki) * 128:(4 + ki) * 128] = w3f[:, :, ki, 1].T
        pr[64:128, (3 + ki) * 128:(4 + ki) * 128] = w3f[:, :, ki, 2].T
        sg[0:64, ki * 128:(ki + 1) * 128] = w3f[:, :, ki, 2].T
        sg[64:128, ki * 128:(ki + 1) * 128] = w3f[:, :, ki, 0].T

    wfq = np.ascontiguousarray(
        inputs["wf"].astype(f).reshape(100, 128, 484).transpose(1, 2, 0)
    ).reshape(128, 484 * 100)

    return {
        "w1sb": w1sb.astype(NPDT),
        "w2ab": w2ab.astype(NPDT),
        "w3p": pr.astype(NPDT), "w3s": sg.astype(NPDT),
        "b1v": np.tile(b1f, 2)[:, None].astype(f),
        "b2v": np.tile(b2f, 2)[:, None].astype(f),
        "b3v": b3f[:, None].astype(f),
        "bfv": inputs["bf"].astype(f)[:, None],
        "wfq": wfq.astype(NPDT),
    }


def prep_core(x, c):
    """Per-core input: x-pixel-on-partition patches, 3 y-shifted replicas."""
    xs = np.asarray(x)[c * BC:(c + 1) * BC, 0].astype(np.float32)  # (16,84,84)
    xr = xs.reshape(BC, 3, 28, 3, 28).transpose(4, 0, 1, 3, 2)     # (x,b,hb,wb,y)
    xrf = xr.reshape(28, P, 28)
    xt = np.zeros((128, P, 28), NPDT)
    for r in range(3):
        xt[32 * r:32 * r + 28, :, 0:28 - r] = xrf[:, :, r:].astype(NPDT)
    return {"xt": xt.reshape(128, P * 28)}


def bench(inputs, iters=8, repeat=192):
    """Measure per-iteration HW time by running the kernel body `repeat`
    times inside one dispatch (tc.For_i) and comparing against repeat=1.
    The ~60-80ms axon dispatch overhead cancels in the difference.
    """
    t1 = _bench_one(inputs, iters, 1)
    tr = _bench_one(inputs, iters, repeat)
    return (tr - t1) / (repeat - 1)


def _bench_one(inputs, iters, repeat):
    import time as _time
    import jax
    from jax.sharding import Mesh, PartitionSpec, NamedSharding
    from jax.experimental.shard_map import shard_map
    from concourse import mybir as _mb
    from concourse import bass2jax

    inputs = {k: np.asarray(v) for k, v in inputs.items()}
    shared = prep_shared(inputs)
    in_maps = [{**shared, **prep_core(inputs["x"], c)} for c in range(NCORES)]
    nc = build_program(repeat=repeat)
    bass2jax.install_neuronx_cc_hook()

    partition_name = nc.partition_id_tensor.name if nc.partition_id_tensor else None
    in_names, out_names, out_avals, zero_outs = [], [], [], []
    for alloc in nc.m.functions[0].allocations:
        if not isinstance(alloc, _mb.MemoryLocationSet):
            continue
        name = alloc.memorylocations[0].name
        if alloc.kind == "ExternalInput":
            if name != partition_name:
                in_names.append(name)
        elif alloc.kind == "ExternalOutput":
            shape = tuple(alloc.tensor_shape)
            dtype = _mb.dt.np(alloc.dtype)
            out_names.append(name)
            out_avals.append(jax.core.ShapedArray(shape, dtype))
            zero_outs.append(np.zeros(shape, dtype))
    n_params = len(in_names)
    all_names = in_names + out_names
    if partition_name is not None:
        all_names = all_names + [partition_name]
    donate = tuple(range(n_params, n_params + len(out_names)))

    def _body(*args):
        operands = list(args)
        if partition_name is not None:
            operands.append(bass2jax.partition_id_tensor())
        outs = bass2jax._bass_exec_p.bind(
            *operands,
            out_avals=tuple(out_avals),
            in_names=tuple(all_names),
            out_names=tuple(out_names),
            lowering_input_output_aliases=(),
            sim_require_finite=True,
            sim_require_nnan=True,
            nc=nc,
        )
        return tuple(outs)

    devices = jax.devices()[:NCORES]
    mesh = Mesh(np.asarray(devices), ("core",))
    spec = NamedSharding(mesh, PartitionSpec("core"))
    sharded = jax.jit(
        shard_map(_body, mesh=mesh,
                  in_specs=(PartitionSpec("core"),) * (n_params + len(out_names)),
                  out_specs=(PartitionSpec("core"),) * len(out_names),
                  check_rep=False),
        donate_argnums=donate, keep_unused=True)

    concat_in = [
        jax.device_put(
            np.concatenate([np.asarray(in_maps[c][n]) for c in range(NCORES)], axis=0),
            spec)
        for n in in_names
    ]

    def _zeros():
        return [jax.device_put(np.zeros((NCORES * z.shape[0], *z.shape[1:]), z.dtype), spec)
                for z in zero_outs]

    r = sharded(*concat_in, *_zeros())   # compile + warm
    jax.block_until_ready(r)
    times = []
    for _ in range(iters):
        zs = _zeros()
        jax.block_until_ready(zs)
        t0 = _time.perf_counter()
        r = sharded(*concat_in, *zs)
        jax.block_until_ready(r)
        times.append(_time.perf_counter() - t0)
    return min(times) * 1e9


def kernel(**inputs):
    global LAST_EXEC_TIME_NS, LAST_PROFILE
    inputs = {k: np.asarray(v) for k, v in inputs.items()}
    shared = prep_shared(inputs)
    in_maps = [{**shared, **prep_core(inputs["x"], c)} for c in range(NCORES)]
    nc = build_program()
    trace = bool(os.environ.get("BASS_KERNEL_TRACE"))
    res = run_bass_kernel_spmd(nc, in_maps, list(range(NCORES)), trace=trace)
    LAST_EXEC_TIME_NS = res.exec_time_ns
    LAST_PROFILE = res.profile_json
    outs = [
        np.asarray(res.results[c]["out"]).T.reshape(BC, 3, 3, 100)
        for c in range(NCORES)
    ]
    return np.concatenate(outs, axis=0)
